# revision 42
# baseline (speedup 1.0000x reference)
"""GAT (2-layer, PyG-style) on 8 Trainium2 NeuronCores — full on-device pipeline.

Structure (host does only data movement / layout):
  L-A (node-sharded): x = relu(feats @ W + b), a1 = x @ [U1|V1] per shard.
  L-B (dst-sharded): layer-1 edge phase.  Edges(+self loops) sorted by dst,
      fixed-span chunks (s dsts, 128 edge slots).  Per chunk: int16 dma_gather
      of [x | a_src] rows from a per-core RELABELED table (unique srcs <
      32767), a_dst and 1/den expanded from local slices via mask matmuls on
      the PE, segment softmax, z = X^T@S selector-matmul SpMM, then
      out1 = z@W1+b1 (relu), h2 = x1@W2, a2 = h2@[u2|v2].
  L-C (dst-sharded): layer-2 edge phase on h2 (heads=1) + output MLP.

Perf backlog (cost-model times: proj 31.2us, gat1 177.0us, gat2 99.4us;
gat1 is PE-bound at 96% occupancy, ~520 tiny den/rde matmuls ~= 70us):
  * Paired rde (worked out, unimplemented): halve the 261 rde matmuls by
    stacking chunk pairs block-diagonally.  Host uploads mdm2[0:s, p*128+e]
    = mdm[2p], mdm2[32:32+s, ...] = mdm[2p+1] (K=56 rows); den for odd
    chunks written at PSUM base_partition 32 via tile_position=(0, 32)
    (documented-legal col-tiling); recip+copy produce a block-diagonal rd
    tile [64 x 2H] (memset once per group for the zero blocks); one matmul
    lhsT=mdm2-pair, rhs=rd-pair -> out [128 x 2H] = [rde_2p | rde_2p+1].
    Est -20us PE on gat1, similar ratio on gat2.
  * den col-tiling (M=24<=32): 4 chunks concurrent via tile_position
    (0, 32j).  NOT modeled by the cost model — verify on HW only.
  * Rejected with evidence: merged src+dst gather (one call/group) loses
    ~30us on gat2 — the monolithic transfer stalls the alpha chain;
    keep the two pipelined gathers.  partition_broadcast APs on DVE are
    rejected by the backend (nonzero-step assert), blocking per-column
    1/den scaling.  All gather variants cost ~22.8ns/descriptor under the
    <512B latency multiplier; only descriptor COUNT matters.
"""
import os
import numpy as np
import ml_dtypes

BF = ml_dtypes.bfloat16
N_NODES = 10000
N_TOTAL = 50000
NC = 8
SH = N_TOTAL // NC          # 6250 dsts per core
SHP = 6272
NEG = 0.2
TBW = 256                   # gather row: [x(128) | a_src(8) | pad] bf16
U_ROWS = 31488              # relabeled table rows (>= max unique srcs + pad)

_cache = {}
_exec_ns = []
_last_launches = []


# ----------------------------------------------------------------- host prep
def _pack_edges(edges):
    """Sort by dst, shard by dst range, fixed-span chunks with 128 slots."""
    src = np.concatenate([np.asarray(edges[0], np.int64),
                          np.arange(N_TOTAL, dtype=np.int64)])
    dst = np.concatenate([np.asarray(edges[1], np.int64),
                          np.arange(N_TOTAL, dtype=np.int64)])
    order = np.argsort(dst, kind="stable")
    src, dst = src[order], dst[order]
    cnt = np.bincount(dst, minlength=N_TOTAL)

    s_pick = None
    for s in (32, 28, 24, 20, 16, 12, 8, 4):
        C = (SH + s - 1) // s
        npad_last = C * s - SH
        ok = True
        for c in range(NC):
            cc = np.zeros(C * s, np.int64)
            cc[:SH] = cnt[c * SH:(c + 1) * SH]
            cc[SH:] = 1                       # virtual pad dsts get one slot
            if cc.reshape(C, s).sum(1).max() > 128:
                ok = False
                break
        if ok:
            s_pick = s
            break
    assert s_pick is not None
    s, C = s_pick, (SH + s_pick - 1) // s_pick
    dstart = np.zeros(N_TOTAL + 1, np.int64)
    np.cumsum(cnt, out=dstart[1:])

    def wrap16(flat):
        NI = len(flat)
        iw = np.zeros((16, NI // 16), np.int16)
        iw[np.arange(NI) % 16, np.arange(NI) // 16] = flat
        return np.ascontiguousarray(np.tile(iw, (8, 1)))

    per_core = []
    for c in range(NC):
        lo = c * SH
        gsrc = np.zeros((C, 128), np.int64)   # global src per slot
        ldst = np.zeros((C, 128), np.int16)   # local dst per slot
        mem = np.zeros((128, SHP), BF)
        mdm = np.zeros((32, C * 128), BF)
        for k in range(C):
            w0 = lo + k * s
            w1r = min(w0 + s, lo + SH)        # real dst end
            e0, e1 = dstart[w0], dstart[w1r]
            n = e1 - e0
            p = np.arange(n)
            gsrc[k, :n] = src[e0:e1]
            ld = dst[e0:e1] - w0
            ldst[k, :n] = k * s + ld
            mem[p, k * s + ld] = 1
            mdm[ld, k * 128 + p] = 1
            nv = (w0 + s) - w1r               # virtual pad dsts (last chunk)
            if nv > 0:
                q = np.arange(n, n + nv)
                vld = np.arange(w1r - w0, s)
                ldst[k, q] = k * s + vld
                mem[q, k * s + vld] = 1
                mdm[vld, k * 128 + q] = 1
        # relabel srcs to per-core local ids (int16 range)
        uniq, inv = np.unique(gsrc.ravel(), return_inverse=True)
        assert len(uniq) <= U_ROWS - 1 and len(uniq) <= 32400
        idx16 = wrap16(inv.astype(np.int16))
        idxd16 = wrap16(ldst.ravel())
        per_core.append((uniq, idx16, idxd16, mem, mdm))
    return s, C, per_core


def _adt_pack(aT_local, H):
    """local dst-side a values [H x SH] -> gather table [SHP, 128] bf16."""
    out = np.zeros((SHP, 128), BF)
    out[:SH, 0:H] = aT_local.T.astype(BF)
    return out


# ------------------------------------------------------------- bass programs
def _build_proj():
    import concourse.bacc as bacc
    import concourse.tile as tile
    import concourse.mybir as mybir
    from contextlib import ExitStack

    F32, BF16 = mybir.dt.float32, mybir.dt.bfloat16
    AF = mybir.ActivationFunctionType
    nc = bacc.Bacc("TRN2", target_bir_lowering=False, debug=False,
                   num_devices=NC)
    nT = nc.dram_tensor("nT", [17, SHP], BF16, kind="ExternalInput")
    cT = nc.dram_tensor("cT", [33, SHP], BF16, kind="ExternalInput")
    wn = nc.dram_tensor("wn", [17, 128], BF16, kind="ExternalInput")
    wc = nc.dram_tensor("wc", [33, 128], BF16, kind="ExternalInput")
    uv = nc.dram_tensor("uv", [128, 16], BF16, kind="ExternalInput")
    xT = nc.dram_tensor("xT", [128, SHP], BF16, kind="ExternalOutput")
    aT = nc.dram_tensor("aT", [16, SHP], F32, kind="ExternalOutput")

    with ExitStack() as ctx:
        tc = ctx.enter_context(tile.TileContext(nc))
        wp = ctx.enter_context(tc.tile_pool(name="w", bufs=1))
        dp = ctx.enter_context(tc.tile_pool(name="d", bufs=3))
        pp = ctx.enter_context(tc.tile_pool(name="p", bufs=2, space="PSUM"))

        wn_t = wp.tile([17, 128], BF16)
        nc.sync.dma_start(wn_t[:], wn.ap())
        wc_t = wp.tile([33, 128], BF16)
        nc.sync.dma_start(wc_t[:], wc.ap())
        uv_t = wp.tile([128, 16], BF16)
        nc.sync.dma_start(uv_t[:], uv.ap())
        xT_sb = wp.tile([128, SHP], BF16)
        aT_sb = wp.tile([16, SHP], F32)

        for g2 in range(SHP // 896):
            o2 = g2 * 896
            nt = dp.tile([17, 896], BF16, tag="nt")
            nc.sync.dma_start(nt[:], nT.ap()[:, o2:o2 + 896])
            ct = dp.tile([33, 896], BF16, tag="ct")
            nc.sync.dma_start(ct[:], cT.ap()[:, o2:o2 + 896])
            for h in range(2):
                o = o2 + h * 448
                sl = slice(h * 448, (h + 1) * 448)
                ps = pp.tile([128, 448], F32, tag="xps")
                nc.tensor.matmul(ps[:], wn_t[:], nt[:, sl],
                                 start=True, stop=False)
                nc.tensor.matmul(ps[:], wc_t[:], ct[:, sl],
                                 start=False, stop=True)
                nc.scalar.activation(xT_sb[:, o:o + 448], ps[:], AF.Relu)
                aps = pp.tile([16, 448], F32, tag="aps")
                nc.tensor.matmul(aps[:], uv_t[:], xT_sb[:, o:o + 448],
                                 start=True, stop=True)
                nc.vector.tensor_copy(aT_sb[:, o:o + 448], aps[:])
        nc.sync.dma_start(xT.ap(), xT_sb[:])
        nc.sync.dma_start(aT.ap(), aT_sb[:])
    nc.compile()
    return nc


def _build_edge(s, C, heads):
    import concourse.bacc as bacc
    import concourse.tile as tile
    import concourse.mybir as mybir
    from concourse.library_config import mlp as mlp_lib
    from contextlib import ExitStack

    F32, BF16, I16 = mybir.dt.float32, mybir.dt.bfloat16, mybir.dt.int16
    AF = mybir.ActivationFunctionType
    OP = mybir.AluOpType
    H = heads
    Gc = 512 // s                              # chunks per group
    slotP = 256 if H == 8 else 32              # psum cols per chunk slot
    BP = 1024 // slotP                         # chunks per (2-bank) psum batch
    SV = C * s                                 # virtual dst count

    nc = bacc.Bacc("TRN2", target_bir_lowering=False, debug=False,
                   num_devices=NC)
    tbl = nc.dram_tensor("tbl", [U_ROWS, TBW], BF16, kind="ExternalInput")
    adt = nc.dram_tensor("adt", [SHP, 128], BF16, kind="ExternalInput")
    idx = nc.dram_tensor("idx", [128, C * 8], I16, kind="ExternalInput")
    idxd = nc.dram_tensor("idxd", [128, C * 8], I16, kind="ExternalInput")
    mem_d = nc.dram_tensor("mem", [128, SHP], BF16, kind="ExternalInput")
    mdm_d = nc.dram_tensor("mdm", [32, C * 128], BF16, kind="ExternalInput")
    if H == 8:
        w1_d = nc.dram_tensor("w1", [128, 1024], BF16, kind="ExternalInput")
        w2_d = nc.dram_tensor("w2", [128, 1024], BF16, kind="ExternalInput")
        uv2_d = nc.dram_tensor("uv2", [128, 2], BF16, kind="ExternalInput")
        b1_d = nc.dram_tensor("b1", [128, 8], F32, kind="ExternalInput")
        h2T_d = nc.dram_tensor("h2T", [128, SHP], BF16,
                               kind="ExternalOutput")
        a2T_d = nc.dram_tensor("a2T", [2, SHP], F32, kind="ExternalOutput")
    else:
        b2_d = nc.dram_tensor("b2", [128, 1], F32, kind="ExternalInput")
        wo1_d = nc.dram_tensor("wo1", [128, 64], BF16, kind="ExternalInput")
        wo2_d = nc.dram_tensor("wo2", [64, 1], BF16, kind="ExternalInput")
        bo1_d = nc.dram_tensor("bo1", [64, 1], F32, kind="ExternalInput")
        bo2_d = nc.dram_tensor("bo2", [1, 1], F32, kind="ExternalInput")
        lgT_d = nc.dram_tensor("lgT", [1, SHP], F32, kind="ExternalOutput")

    with ExitStack() as ctx:
        tc = ctx.enter_context(tile.TileContext(nc))
        wp = ctx.enter_context(tc.tile_pool(name="w", bufs=1))
        gp = ctx.enter_context(tc.tile_pool(name="g", bufs=2))
        sp = ctx.enter_context(tc.tile_pool(name="s", bufs=2))
        tp = ctx.enter_context(tc.tile_pool(name="t", bufs=3))
        zp = ctx.enter_context(tc.tile_pool(name="zp", bufs=2, space="PSUM"))
        ep = ctx.enter_context(tc.tile_pool(name="ep", bufs=1, space="PSUM"))
        gpp = ctx.enter_context(tc.tile_pool(name="gpp", bufs=2,
                                             space="PSUM"))

        nc.gpsimd.load_library(mlp_lib)
        mem = wp.tile([128, SHP], BF16)
        nc.sync.dma_start(mem[:], mem_d.ap())
        it = wp.tile([128, C * 8], I16)
        nc.sync.dma_start(it[:], idx.ap())
        itd = wp.tile([128, C * 8], I16)
        nc.sync.dma_start(itd[:], idxd.ap())
        if H == 8:
            w1 = wp.tile([128, 1024], BF16)
            nc.sync.dma_start(w1[:], w1_d.ap())
            w2 = wp.tile([128, 1024], BF16)
            nc.sync.dma_start(w2[:], w2_d.ap())
            uv2 = wp.tile([128, 2], BF16)
            nc.sync.dma_start(uv2[:], uv2_d.ap())
            b1 = wp.tile([128, 8], F32)
            nc.sync.dma_start(b1[:], b1_d.ap())
            h2T = wp.tile([128, SHP], BF16)
            a2T = wp.tile([2, SHP], F32)
            if C * s < SHP:
                nc.gpsimd.memset(h2T[:, C * s:SHP], 0.0)
                nc.gpsimd.memset(a2T[:, C * s:SHP], 0.0)
        else:
            b2 = wp.tile([128, 1], F32)
            nc.sync.dma_start(b2[:], b2_d.ap())
            wo1 = wp.tile([128, 64], BF16)
            nc.sync.dma_start(wo1[:], wo1_d.ap())
            wo2 = wp.tile([64, 1], BF16)
            nc.sync.dma_start(wo2[:], wo2_d.ap())
            bo1 = wp.tile([64, 1], F32)
            nc.sync.dma_start(bo1[:], bo1_d.ap())
            bo2 = wp.tile([1, 1], F32)
            nc.sync.dma_start(bo2[:], bo2_d.ap())
            lgT = wp.tile([1, SHP], F32)
            if C * s < SHP:
                nc.gpsimd.memset(lgT[:, C * s:SHP], 0.0)

        ngroups = (C + Gc - 1) // Gc
        for g in range(ngroups):
            k0 = g * Gc
            ng = min(Gc, C - k0)
            o0 = k0 * s

            xg = gp.tile([128, Gc, TBW], BF16, tag="xg")
            nc.gpsimd.dma_gather(xg[:, 0:ng, :], tbl.ap(),
                                 it[:, k0 * 8:(k0 + ng) * 8],
                                 ng * 128, ng * 128, TBW,
                                 single_packet=False)
            mdm = gp.tile([32, Gc * 128], BF16, tag="mdm")
            nc.sync.dma_start(mdm[:, 0:ng * 128],
                              mdm_d.ap()[:, k0 * 128:(k0 + ng) * 128])
            adg = gp.tile([128, Gc, 128], BF16, tag="adg")
            nc.gpsimd.dma_gather(adg[:, 0:ng, :], adt.ap(),
                                 itd[:, k0 * 8:(k0 + ng) * 8],
                                 ng * 128, ng * 128, 128,
                                 single_packet=False)

            # alpha = exp(lrelu(asrc + adst)) = max(exp(a), exp(0.2 a))
            alr = tp.tile([128, Gc, H], F32, tag="alr")
            nc.vector.tensor_tensor(
                out=alr[:, 0:ng, :], in0=xg[:, 0:ng, 128:128 + H],
                in1=adg[:, 0:ng, 0:H], op=OP.add)
            e1 = tp.tile([128, Gc, H], F32, tag="e1")
            nc.scalar.activation(e1[:, 0:ng, :], alr[:, 0:ng, :], AF.Exp)
            e2 = tp.tile([128, Gc, H], F32, tag="e2")
            nc.scalar.activation(e2[:, 0:ng, :], alr[:, 0:ng, :], AF.Exp,
                                 scale=NEG)
            ex = tp.tile([128, Gc, H], BF16, tag="ex")
            nc.vector.tensor_tensor(out=ex[:, 0:ng, :], in0=e1[:, 0:ng, :],
                                    in1=e2[:, 0:ng, :], op=OP.max)

            # den + reciprocal + expansion back to edges
            den = ep.tile([32, Gc * H], F32, tag="den")
            for j in range(ng):
                nc.tensor.matmul(den[0:s, j * H:(j + 1) * H],
                                 mem[:, (k0 + j) * s:(k0 + j) * s + s],
                                 ex[:, j, :], start=True, stop=True)
            rdf = tp.tile([32, Gc * H], F32, tag="rdf")
            nc.vector.reciprocal(rdf[0:s, 0:ng * H], den[0:s, 0:ng * H])
            rd = tp.tile([32, Gc * H], BF16, tag="rd")
            nc.vector.tensor_copy(rd[0:s, 0:ng * H], rdf[0:s, 0:ng * H])
            rde = ep.tile([128, Gc * H], F32, tag="rde")
            for j in range(ng):
                nc.tensor.matmul(rde[:, j * H:(j + 1) * H],
                                 mdm[0:s, j * 128:(j + 1) * 128],
                                 rd[0:s, j * H:(j + 1) * H],
                                 start=True, stop=True)
            alf = tp.tile([128, Gc, H], BF16, tag="alf")
            nc.vector.tensor_tensor(
                out=alf[:, 0:ng, :], in0=ex[:, 0:ng, :],
                in1=rde[:, 0:ng * H].rearrange("p (b h) -> p b h", b=ng),
                op=OP.mult)

            # S = mask * alpha  (one op), then SpMM per chunk
            S = sp.tile([128, Gc, H, s], BF16, tag="S")
            nc.vector.tensor_tensor(
                out=S[:, 0:ng, :, :],
                in0=mem[:, o0:o0 + ng * s].rearrange(
                    "p (b j) -> p b j", b=ng).unsqueeze(2).to_broadcast(
                    [128, ng, H, s]),
                in1=alf[:, 0:ng, :].unsqueeze(3).to_broadcast(
                    [128, ng, H, s]),
                op=OP.mult)

            if H == 8:
                zg = sp.tile([128, Gc, H, s], BF16, tag="zg")
            else:
                x2g = sp.tile([128, Gc, s], BF16, tag="x2g")
            nb = (ng + BP - 1) // BP
            for b in range(nb):
                j0 = b * BP
                nj = min(BP, ng - j0)
                zb = zp.tile([128, BP, slotP], F32, tag="zb")
                for j in range(j0, j0 + nj):
                    nc.tensor.matmul(zb[:, j - j0, 0:H * s],
                                     xg[:, j, 0:128], S[:, j, :, :],
                                     start=True, stop=True)
                if H == 8:
                    src = zb[:, 0:nj, 0:H * s].rearrange(
                        "p b (h j) -> p b h j", h=H)
                    if b % 2 == 0:
                        nc.vector.tensor_copy(zg[:, j0:j0 + nj, :, :], src)
                    else:
                        nc.scalar.activation(zg[:, j0:j0 + nj, :, :], src,
                                             AF.Copy)
                else:
                    nc.scalar.activation(x2g[:, j0:j0 + nj, :],
                                         zb[:, 0:nj, 0:s],
                                         AF.Relu, bias=b2[:, 0:1])

            if H == 8:
                x1g = sp.tile([128, 8, Gc * s], BF16, tag="x1g")
                for h in range(8):
                    o1 = gpp.tile([128, Gc * s], F32, tag="gp")
                    nc.tensor.matmul(o1[:, 0:ng * s],
                                     w1[:, h * 128:(h + 1) * 128],
                                     zg[:, 0:ng, h, :],
                                     start=True, stop=True)
                    nc.scalar.activation(x1g[:, h, 0:ng * s],
                                         o1[:, 0:ng * s], AF.Relu,
                                         bias=b1[:, h:h + 1])
                h2p = gpp.tile([128, Gc * s], F32, tag="gp")
                for k in range(8):
                    nc.tensor.matmul(h2p[:, 0:ng * s],
                                     w2[:, k * 128:(k + 1) * 128],
                                     x1g[:, k, 0:ng * s],
                                     start=(k == 0), stop=(k == 7))
                nc.vector.tensor_copy(h2T[:, o0:o0 + ng * s],
                                      h2p[:, 0:ng * s])
                a2p = gpp.tile([2, Gc * s], F32, tag="gp")
                nc.tensor.matmul(a2p[:, 0:ng * s], uv2[:],
                                 h2T[:, o0:o0 + ng * s],
                                 start=True, stop=True)
                nc.vector.tensor_copy(a2T[:, o0:o0 + ng * s],
                                      a2p[:, 0:ng * s])
            else:
                mp = gpp.tile([64, Gc * s], F32, tag="gp")
                nc.tensor.matmul(mp[:, 0:ng * s], wo1[:], x2g[:, 0:ng, :],
                                 start=True, stop=True)
                mT = tp.tile([64, Gc * s], BF16, tag="mT")
                nc.scalar.activation(mT[:, 0:ng * s], mp[:, 0:ng * s],
                                     AF.Relu, bias=bo1[:, 0:1])
                lp = gpp.tile([1, Gc * s], F32, tag="gp")
                nc.tensor.matmul(lp[:, 0:ng * s], wo2[:], mT[:, 0:ng * s],
                                 start=True, stop=True)
                nc.vector.tensor_scalar_add(lgT[:, o0:o0 + ng * s],
                                            lp[:, 0:ng * s],
                                            bo2[0:1, 0:1])

        if H == 8:
            nc.sync.dma_start(h2T_d.ap(), h2T[:])
            nc.sync.dma_start(a2T_d.ap(), a2T[:])
        else:
            nc.sync.dma_start(lgT_d.ap(), lgT[:])
    nc.compile()
    return nc


# ------------------------------------------------------------------- runner
def _run(nc, maps, label):
    import time
    from concourse.bass_utils import run_bass_kernel_spmd
    _last_launches.append((label, nc, maps))
    t0 = time.perf_counter()
    res = run_bass_kernel_spmd(nc, maps, core_ids=list(range(NC)))
    t1 = time.perf_counter()
    _exec_ns.append((label, int((t1 - t0) * 1e9)))
    return res.results


def kernel(node_features, column_features, edges, node_num,
           Wn, bn, Wc, bc, W1, att_src1, att_dst1, b1,
           W2, att_src2, att_dst2, b2, Wo1, bo1, Wo2, bo2):
    del node_num
    f32 = np.float32
    node_features = np.asarray(node_features, f32)
    column_features = np.asarray(column_features, f32)
    edges = np.asarray(edges)
    Wn, bn, Wc, bc = (np.asarray(a, f32) for a in (Wn, bn, Wc, bc))
    W1, att_src1, att_dst1, b1 = (np.asarray(a, f32) for a in
                                  (W1, att_src1, att_dst1, b1))
    W2, att_src2, att_dst2, b2 = (np.asarray(a, f32) for a in
                                  (W2, att_src2, att_dst2, b2))
    Wo1, bo1, Wo2, bo2 = (np.asarray(a, f32) for a in (Wo1, bo1, Wo2, bo2))
    _exec_ns.clear()
    _last_launches.clear()

    s, C, per_core = _pack_edges(edges)
    key = (s, C)
    if key not in _cache:
        _cache[key] = (_build_proj(), _build_edge(s, C, 8),
                       _build_edge(s, C, 1))
    ncA, ncB, ncC = _cache[key]

    # ---------------- L-A
    wn_p = np.concatenate([Wn, bn[None, :]], 0).astype(BF)
    wc_p = np.concatenate([Wc, bc[None, :]], 0).astype(BF)
    W1r = W1.reshape(128, 8, 128)
    uv1 = np.concatenate([
        np.einsum("ihd,hd->ih", W1r, att_src1),
        np.einsum("ihd,hd->ih", W1r, att_dst1)], 1).astype(BF)  # [128,16]
    mapsA = []
    for c in range(NC):
        rows = np.arange(c * SH, (c + 1) * SH)
        nT = np.zeros((17, SHP), BF)
        cT = np.zeros((33, SHP), BF)
        nmask = rows < N_NODES
        if nmask.any():
            ii = np.where(nmask)[0]
            nT[0:16, ii] = node_features[rows[ii]].T.astype(BF)
            nT[16, ii] = 1
        if (~nmask).any():
            ii = np.where(~nmask)[0]
            cT[0:32, ii] = column_features[rows[ii] - N_NODES].T.astype(BF)
            cT[32, ii] = 1
        mapsA.append({"nT": nT, "cT": cT, "wn": wn_p, "wc": wc_p, "uv": uv1})
    resA = _run(ncA, mapsA, "proj")
    xT = np.concatenate([resA[c]["xT"][:, :SH] for c in range(NC)], 1)
    aT = np.concatenate([resA[c]["aT"][:, :SH] for c in range(NC)], 1)

    tblG = np.zeros((N_TOTAL, TBW), BF)
    tblG[:, 0:128] = xT.T
    tblG[:, 128:136] = aT[0:8].T.astype(BF)

    # ---------------- L-B
    w2p = np.ascontiguousarray(
        W2.reshape(8, 128, 128).transpose(1, 0, 2)).reshape(128, 1024)
    uv2 = np.concatenate([att_src2.T, att_dst2.T], 1).astype(BF)
    mapsB = []
    for c in range(NC):
        uniq, idx16, idxd16, mem_, mdm_ = per_core[c]
        tblL = np.zeros((U_ROWS, TBW), BF)
        tblL[:len(uniq)] = tblG[uniq]
        mapsB.append({
            "tbl": tblL, "idx": idx16, "idxd": idxd16, "mem": mem_,
            "mdm": mdm_,
            "adt": _adt_pack(aT[8:16, c * SH:(c + 1) * SH], 8),
            "w1": W1.astype(BF), "w2": w2p.astype(BF), "uv2": uv2,
            "b1": np.ascontiguousarray(b1.reshape(8, 128).T).astype(f32),
        })
    resB = _run(ncB, mapsB, "gat1")
    h2T = np.concatenate([resB[c]["h2T"][:, :SH] for c in range(NC)], 1)
    a2T = np.concatenate([resB[c]["a2T"][:, :SH] for c in range(NC)], 1)

    tblG2 = np.zeros((N_TOTAL, TBW), BF)
    tblG2[:, 0:128] = h2T.T
    tblG2[:, 128] = a2T[0].astype(BF)

    # ---------------- L-C
    mapsC = []
    for c in range(NC):
        uniq, idx16, idxd16, mem_, mdm_ = per_core[c]
        tblL = np.zeros((U_ROWS, TBW), BF)
        tblL[:len(uniq)] = tblG2[uniq]
        mapsC.append({
            "tbl": tblL, "idx": idx16, "idxd": idxd16, "mem": mem_,
            "mdm": mdm_,
            "adt": _adt_pack(a2T[1:2, c * SH:(c + 1) * SH], 1),
            "b2": b2.reshape(128, 1).astype(f32),
            "wo1": Wo1.astype(BF), "wo2": Wo2.astype(BF),
            "bo1": bo1.reshape(64, 1).astype(f32),
            "bo2": bo2.reshape(1, 1).astype(f32),
        })
    resC = _run(ncC, mapsC, "gat2")
    lg = np.concatenate([resC[c]["lgT"][0, :SH] for c in range(NC)])
    return lg[:N_NODES].astype(np.float32)
